# revision 1
# baseline (speedup 1.0000x reference)
"""Deformable conv (DeformConvPack) Bass kernel for 8 Trainium2 NeuronCores.

Problem (hardcoded): x[4,64,128,256] f32, offset[4,18,128,256] f32,
weight[64,64,3,3] f32, bias[64] f32 -> out[4,64,128,256] f32.
stride=1, pad=1, dil=1, deformable_groups=1.

Sharding: 8 cores = batch(4) x W-halves(2). Core c: b=c//2, w0=(c%2)*128.

Per-core pipeline:
  - Host builds a zero-padded channels-last "4-corner" table per core
    (W-slice + halo):  T[y, x, yp, c] = x[b, c, y-PAD, w0 + x-PAD + ...]
    flattened to rows r = y*TX+x of 128 bf16; one gather descriptor fetches
    rows r, r+1 = all 4 bilinear corners for all 64 channels (512B).
  - Device computes per tap: sample coords, floor/frac, corner weights, and
    int16 row indices (DVE); PE double-transpose wraps indices into the
    16-partition replicated layout dma_gather requires.
  - dma_gather gathers; DVE combines 4 corners with per-partition scalar
    MACs; PE transposes [pos,c]->[c,pos]; ACT copies PSUM->SBUF; PE does the
    (k,c)-contracted conv as PSUM-accumulated matmuls; bias add; DMA out.
"""

import numpy as np
import ml_dtypes

B, C, H, W = 4, 64, 128, 256
Cout, kH, kW = 64, 3, 3
K = kH * kW
WH = 128          # per-core W slice
PAD = 12          # table padding (rows and cols, each side)
TY = H + 2 * PAD            # 152
TX = WH + 2 * PAD           # 152
TROWS = TY * TX             # 23104
NI = 128          # i (wo within slice) range per core
NP = 128          # partitions = ho
CHUNK = 16        # i-cols per gather call
NCHUNK = NI // CHUNK

_CACHE = {}


def _build_bass():
    import concourse.bacc as bacc
    import concourse.mybir as mybir
    from concourse import bass
    from concourse.tile import TileContext
    from concourse.masks import make_identity

    f32 = mybir.dt.float32
    i16 = mybir.dt.int16
    bf16 = mybir.dt.bfloat16

    nc = bacc.Bacc(None, target_bir_lowering=False)

    tbl = nc.declare_dram_parameter("tbl", [TROWS, 128], bf16, isOutput=False)
    off = nc.declare_dram_parameter("off", [K, 2, NP, NI], f32, isOutput=False)
    w2 = nc.declare_dram_parameter("w2", [5, 128, Cout], bf16, isOutput=False)
    biasp = nc.declare_dram_parameter("bias", [Cout, 1], f32, isOutput=False)
    rowbase = nc.declare_dram_parameter("rowbase", [NP, 3], f32, isOutput=False)
    colbase = nc.declare_dram_parameter("colbase", [3, NP, NI], f32, isOutput=False)
    outp = nc.declare_dram_parameter("out", [Cout, NP * NI], f32, isOutput=True)

    mult = mybir.AluOpType.mult
    add = mybir.AluOpType.add
    sub = mybir.AluOpType.subtract
    is_gt = mybir.AluOpType.is_gt
    amin = mybir.AluOpType.min
    amax = mybir.AluOpType.max
    ACopy = mybir.ActivationFunctionType.Copy

    # gather source: overlapping AP over half-rows (stride 128, len 256)
    tbl_src = bass.AP(tbl, 0, [[128, TROWS - 1], [1, 256]])

    with TileContext(nc) as tc:
        with (
            tc.tile_pool(name="const", bufs=1) as cpool,
            tc.tile_pool(name="persist", bufs=1) as ppool,
            tc.tile_pool(name="scratch", bufs=3) as spool,
            tc.tile_pool(name="gather", bufs=2) as gpool,
            tc.tile_pool(name="vals", bufs=1) as vpool,
            tc.tile_pool(name="valt", bufs=2) as tpool,
            tc.tile_pool(name="psum", bufs=1, space="PSUM") as psp,
        ):
            # ---- constants ----
            ident = cpool.tile([128, 128], bf16)
            make_identity(nc, ident[:])
            identf = cpool.tile([128, 128], f32)
            make_identity(nc, identf[:])
            w2sb = cpool.tile([128, 5 * Cout], bf16)
            for g in range(5):
                nc.sync.dma_start(out=w2sb[:, g * Cout:(g + 1) * Cout], in_=w2[g])
            bias_sb = cpool.tile([Cout, 1], f32)
            nc.sync.dma_start(out=bias_sb[:], in_=biasp[:])
            rb = cpool.tile([NP, 3], f32)
            nc.sync.dma_start(out=rb[:], in_=rowbase[:])
            cb = cpool.tile([NP, 3 * NI], f32)
            for kj in range(3):
                nc.sync.dma_start(out=cb[:, kj * NI:(kj + 1) * NI], in_=colbase[kj])

            # ---- per-tap index & weight fields ----
            wrap_k, wy0_k, wx0_k, w01_k, w10_k, w11_k = [], [], [], [], [], []
            for k in range(K):
                ki, kj = k // 3, k % 3
                dy = spool.tile([NP, NI], f32, tag="dy")
                dx = spool.tile([NP, NI], f32, tag="dx")
                nc.sync.dma_start(out=dy[:], in_=off[k, 0])
                nc.sync.dma_start(out=dx[:], in_=off[k, 1])

                py = spool.tile([NP, NI], f32, tag="py")
                px = spool.tile([NP, NI], f32, tag="px")
                # py = dy + (p - 1 + ki + PAD); px = dx + (i - 1 + kj + PAD)
                nc.vector.tensor_scalar(py[:], dy[:], rb[:, ki:ki + 1], None, add)
                nc.vector.tensor_tensor(px[:], dx[:], cb[:, kj * NI:(kj + 1) * NI], add)

                def floor_frac(src, tag):
                    ti = spool.tile([NP, NI], mybir.dt.int32, tag=f"ti{tag}")
                    tf = spool.tile([NP, NI], f32, tag=f"tf{tag}")
                    corr = spool.tile([NP, NI], f32, tag=f"co{tag}")
                    fl = spool.tile([NP, NI], f32, tag=f"fl{tag}")
                    fr = spool.tile([NP, NI], f32, tag=f"fr{tag}")
                    nc.vector.tensor_copy(ti[:], src[:])
                    nc.vector.tensor_copy(tf[:], ti[:])
                    nc.vector.tensor_tensor(corr[:], tf[:], src[:], is_gt)
                    nc.vector.tensor_tensor(fl[:], tf[:], corr[:], sub)
                    nc.vector.tensor_tensor(fr[:], src[:], fl[:], sub)
                    return fl, fr

                y0f, ly = floor_frac(py, "y")
                x0f, lx = floor_frac(px, "x")

                wy0 = ppool.tile([NP, NI], f32, tag=f"wy0_{k}")
                wx0 = ppool.tile([NP, NI], f32, tag=f"wx0_{k}")
                w01 = ppool.tile([NP, NI], f32, tag=f"w01_{k}")
                w10 = ppool.tile([NP, NI], f32, tag=f"w10_{k}")
                w11 = ppool.tile([NP, NI], f32, tag=f"w11_{k}")
                nc.vector.tensor_scalar(wy0[:], ly[:], 1.0, -1.0, sub, mult)
                nc.vector.tensor_scalar(wx0[:], lx[:], 1.0, -1.0, sub, mult)
                nc.vector.tensor_tensor(w01[:], ly[:], wx0[:], mult)   # (xp0,yp1)
                nc.vector.tensor_tensor(w10[:], wy0[:], lx[:], mult)   # (xp1,yp0)
                nc.vector.tensor_tensor(w11[:], ly[:], lx[:], mult)    # (xp1,yp1)

                r1 = spool.tile([NP, NI], f32, tag="r1")
                r2 = spool.tile([NP, NI], f32, tag="r2")
                nc.vector.tensor_scalar(r1[:], y0f[:], float(TX), None, mult)
                nc.vector.tensor_tensor(r2[:], r1[:], x0f[:], add)
                nc.vector.tensor_scalar(r2[:], r2[:], float(TROWS - 2), 0.0, amin, amax)

                # ---- wrap r into dma_gather idx layout ----
                # wrapped[pp+16q, j*8+ph] = r[16ph+pp, j]
                rpsum = psp.tile([128, 128], f32, tag="rpsum", bufs=1)
                nc.tensor.transpose(out=rpsum[:], in_=r2[:], identity=identf[:])
                wrapP = psp.tile([128, 8 * 128], f32, tag="wrapP", bufs=1)
                for ph in range(8):
                    rT_ph = spool.tile([128, 128], f32, tag="rT", bufs=2,
                                       name=f"rT{ph}")
                    nc.vector.tensor_copy(
                        rT_ph[:].rearrange("p (q s) -> p q s", q=8),
                        rpsum[:, None, 16 * ph:16 * (ph + 1)]
                        .to_broadcast([128, 8, 16]))
                    nc.tensor.transpose(
                        out=wrapP[:, 128 * ph:128 * (ph + 1)],
                        in_=rT_ph[:],
                        identity=identf[:])
                wrapped = ppool.tile([128, 1024], i16, tag=f"wrap_{k}")
                nc.vector.tensor_copy(
                    wrapped[:].rearrange("p (j h) -> p j h", h=8),
                    wrapP[:].rearrange("p (h j) -> p j h", h=8))

                wrap_k.append(wrapped)
                wy0_k.append(wy0)
                wx0_k.append(wx0)
                w01_k.append(w01)
                w10_k.append(w10)
                w11_k.append(w11)

            # ---- main loop over i-chunks ----
            out_bf = ppool.tile([Cout, NP, NI], bf16, tag="outbf")  # [o, p, i]
            for cc in range(NCHUNK):
                i0 = cc * CHUNK
                vals = []
                for k in range(K):
                    G = gpool.tile([128, CHUNK * 256], bf16, tag="G")
                    nc.gpsimd.dma_gather(
                        out_ap=G[:].rearrange("p (j e) -> p j e", e=256),
                        in_ap=tbl_src,
                        idxs_ap=wrap_k[k][:, 128 * cc:128 * (cc + 1)],
                        num_idxs=CHUNK * 128,
                        num_idxs_reg=CHUNK * 128,
                        elem_size=256,
                        elem_step=128,
                        single_packet=False,
                    )
                    val = vpool.tile([128, CHUNK * Cout], bf16, tag=f"val{k}")
                    for i in range(CHUNK):
                        ia = i0 + i
                        gof = i * 256
                        vof = i * Cout
                        vsl = val[:, vof:vof + Cout]
                        nc.vector.tensor_scalar(
                            vsl, G[:, gof:gof + 64],
                            wy0_k[k][:, ia:ia + 1], wx0_k[k][:, ia:ia + 1],
                            mult, mult)
                        nc.vector.scalar_tensor_tensor(
                            vsl, G[:, gof + 64:gof + 128],
                            w01_k[k][:, ia:ia + 1], vsl, mult, add)
                        nc.vector.scalar_tensor_tensor(
                            vsl, G[:, gof + 128:gof + 192],
                            w10_k[k][:, ia:ia + 1], vsl, mult, add)
                        nc.vector.scalar_tensor_tensor(
                            vsl, G[:, gof + 192:gof + 256],
                            w11_k[k][:, ia:ia + 1], vsl, mult, add)
                    vals.append(val)

                # transpose + copy to valT
                valts = []
                for g in range(5):
                    vt = tpool.tile([128, CHUNK * 128], bf16, tag=f"vt{g}")
                    valts.append(vt)
                for i in range(CHUNK):
                    for g in range(5):
                        ka, kb = 2 * g, min(2 * g + 1, K - 1)
                        pt = psp.tile([128, 128], bf16, tag="pt", bufs=2)
                        nc.tensor.transpose(
                            out=pt[0:64, :],
                            in_=vals[ka][:, i * Cout:(i + 1) * Cout],
                            identity=ident[:])
                        nc.tensor.transpose(
                            out=pt[64:128, :],
                            in_=vals[kb][:, i * Cout:(i + 1) * Cout],
                            identity=ident[:])
                        nc.scalar.activation(
                            out=valts[g][:, i * 128:(i + 1) * 128],
                            in_=pt[:, :], func=ACopy)

                # conv matmuls + bias
                for isub in range(0, CHUNK, 2):
                    outps = [psp.tile([Cout, 128], f32, tag="op", bufs=3,
                                      name=f"op{t}") for t in range(2)]
                    for g in range(5):
                        for t in range(2):
                            i = isub + t
                            nc.tensor.matmul(
                                out=outps[t][:],
                                lhsT=w2sb[:, g * Cout:(g + 1) * Cout],
                                rhs=valts[g][:, i * 128:(i + 1) * 128],
                                start=(g == 0), stop=(g == 4))
                    for t in range(2):
                        ia = i0 + isub + t
                        nc.vector.tensor_scalar(
                            out_bf[:, :, ia], outps[t][:],
                            bias_sb[:, 0:1], None, add)

            # ---- cast + store ----
            for pc in range(8):
                cbuf = spool.tile([Cout, 16, NI], f32, tag="cast", bufs=2)
                nc.vector.tensor_copy(cbuf[:], out_bf[:, pc * 16:(pc + 1) * 16, :])
                nc.sync.dma_start(
                    out=outp[:, pc * 16 * NI:(pc + 1) * 16 * NI],
                    in_=cbuf[:])

    nc.compile()
    return nc


def _host_prep(x, offset, weight, bias):
    bf16 = ml_dtypes.bfloat16
    # per-core 4-corner tables (batch x W-half, with halo)
    tbls = []
    for core in range(8):
        b, w0 = core // 2, (core % 2) * WH
        T = np.zeros((TY, TX, 2, C), dtype=bf16)
        xlo = max(0, w0 - PAD)
        xhi = min(W, w0 + WH + PAD)
        # table x-col for global x: xt = x - w0 + PAD
        tlo, thi = xlo - w0 + PAD, xhi - w0 + PAD
        xt = np.ascontiguousarray(x[b].transpose(1, 2, 0))  # [H, W, C]
        T[PAD:PAD + H, tlo:thi, 0, :] = xt[:, xlo:xhi]
        T[PAD - 1:PAD - 1 + H, tlo:thi, 1, :] = xt[:, xlo:xhi]
        tbls.append(T.reshape(TROWS, 128))
    # conv weights: W2[g, ks*64+c, o] = weight[o, c, 2g+ks]
    wr = weight.reshape(Cout, C, K)
    w2 = np.zeros((5, 128, Cout), dtype=bf16)
    for g in range(5):
        w2[g, 0:64, :] = wr[:, :, 2 * g].T
        if 2 * g + 1 < K:
            w2[g, 64:128, :] = wr[:, :, 2 * g + 1].T
    biasc = np.ascontiguousarray(bias.reshape(Cout, 1).astype(np.float32))
    rowbase = np.zeros((NP, 3), np.float32)
    for ki in range(3):
        rowbase[:, ki] = np.arange(NP) - 1 + ki + PAD
    colbase = np.zeros((3, NP, NI), np.float32)
    for kj in range(3):
        colbase[kj, :, :] = (np.arange(NI) - 1 + kj + PAD)[None, :]
    return tbls, w2, biasc, rowbase, colbase


def kernel(x, offset, weight, bias):
    from concourse.bass_utils import run_bass_kernel_spmd

    assert float(np.abs(offset).max()) < PAD - 2.0, "offset outside supported band"

    if "nc" not in _CACHE:
        _CACHE["nc"] = _build_bass()
    nc = _CACHE["nc"]

    tbls, w2, biasc, rowbase, colbase = _host_prep(x, offset, weight, bias)

    in_maps = []
    for core in range(8):
        b, w0 = core // 2, (core % 2) * WH
        offs = np.ascontiguousarray(
            offset[b].reshape(K, 2, H, W)[:, :, :, w0:w0 + WH]).astype(np.float32)
        in_maps.append({
            "tbl": tbls[core],
            "off": offs,
            "w2": w2,
            "bias": biasc,
            "rowbase": rowbase,
            "colbase": colbase,
        })

    res = run_bass_kernel_spmd(nc, in_maps, list(range(8)))

    out = np.empty((B, Cout, H, W), np.float32)
    for core in range(8):
        b, w0 = core // 2, (core % 2) * WH
        out[b, :, :, w0:w0 + WH] = res.results[core]["out"].reshape(Cout, NP, NI)
    return out



# revision 7
# speedup vs baseline: 1.6875x; 1.6875x over previous
"""Deformable conv (DeformConvPack) Bass kernel for 8 Trainium2 NeuronCores.

Problem (hardcoded): x[4,64,128,256] f32, offset[4,18,128,256] f32,
weight[64,64,3,3] f32, bias[64] f32 -> out[4,64,128,256] f32.
stride=1, pad=1, dil=1, deformable_groups=1.

Sharding: 8 cores = batch(4) x W-halves(2). Core c: b=c//2, w0=(c%2)*128.

Per-core pipeline:
  - Host builds a zero-padded channels-last "4-corner" table per core
    (W-slice + halo):  T[y, x, yp, c] = x[b, c, y-PAD, w0 + x-PAD + ...]
    flattened to rows r = y*TX+x of 128 bf16; one gather descriptor fetches
    rows r, r+1 = all 4 bilinear corners for all 64 channels (512B).
  - Device computes per tap: sample coords, floor/frac, corner weights, and
    int16 row indices (DVE); PE double-transpose wraps indices into the
    16-partition replicated layout dma_gather requires.
  - dma_gather gathers; DVE combines 4 corners with per-partition scalar
    MACs; PE transposes [pos,c]->[c,pos]; ACT copies PSUM->SBUF; PE does the
    (k,c)-contracted conv as PSUM-accumulated matmuls; bias add; DMA out.
"""

import numpy as np
import ml_dtypes

B, C, H, W = 4, 64, 128, 256
Cout, kH, kW = 64, 3, 3
K = kH * kW
WH = 128          # per-core W slice
PAD = 12          # table padding (rows and cols, each side)
TY = H + 2 * PAD            # 152
TX = WH + 2 * PAD           # 152
TROWS = TY * TX             # 23104
NI = 128          # i (wo within slice) range per core
NP = 128          # partitions = ho
CHUNK = 16        # i-cols per gather call
NCHUNK = NI // CHUNK

_CACHE = {}


def _build_bass():
    import concourse.bacc as bacc
    import concourse.mybir as mybir
    from concourse import bass
    from concourse.tile import TileContext
    from concourse.masks import make_identity

    f32 = mybir.dt.float32
    i16 = mybir.dt.int16
    bf16 = mybir.dt.bfloat16

    nc = bacc.Bacc(None, target_bir_lowering=False, num_swdge_queues=4)

    tbl = nc.declare_dram_parameter("tbl", [TROWS, 128], bf16, isOutput=False)
    off = nc.declare_dram_parameter("off", [K, 2, NP, NI], f32, isOutput=False)
    w2 = nc.declare_dram_parameter("w2", [5, 128, Cout], bf16, isOutput=False)
    biasp = nc.declare_dram_parameter("bias", [Cout, 1], f32, isOutput=False)
    rowbase = nc.declare_dram_parameter("rowbase", [NP, 3], f32, isOutput=False)
    colbase = nc.declare_dram_parameter("colbase", [3, NP, NI], f32, isOutput=False)
    outp = nc.declare_dram_parameter("out", [Cout, NP * NI], f32, isOutput=True)

    mult = mybir.AluOpType.mult
    add = mybir.AluOpType.add
    sub = mybir.AluOpType.subtract
    is_gt = mybir.AluOpType.is_gt
    amin = mybir.AluOpType.min
    amax = mybir.AluOpType.max
    ACopy = mybir.ActivationFunctionType.Copy

    # gather source: overlapping AP over half-rows (stride 128, len 256)
    tbl_src = bass.AP(tbl, 0, [[128, TROWS - 1], [1, 256]])

    with TileContext(nc) as tc:
        with (
            tc.tile_pool(name="const", bufs=1) as cpool,
            tc.tile_pool(name="persist", bufs=1) as ppool,
            tc.tile_pool(name="scratch", bufs=3) as spool,
            tc.tile_pool(name="gather", bufs=2) as gpool,
            tc.tile_pool(name="vals", bufs=1) as vpool,
            tc.tile_pool(name="valt", bufs=2) as tpool,
            tc.tile_pool(name="psum", bufs=1, space="PSUM") as psp,
        ):
            # ---- constants ----
            ident = cpool.tile([128, 128], bf16)
            make_identity(nc, ident[:])
            identf = cpool.tile([128, 128], f32)
            make_identity(nc, identf[:])
            w2sb = cpool.tile([128, 5 * Cout], bf16)
            for g in range(5):
                nc.sync.dma_start(out=w2sb[:, g * Cout:(g + 1) * Cout], in_=w2[g])
            bias_sb = cpool.tile([Cout, 1], f32)
            nc.sync.dma_start(out=bias_sb[:], in_=biasp[:])
            rb = cpool.tile([NP, 3], f32)
            nc.sync.dma_start(out=rb[:], in_=rowbase[:])
            cb = cpool.tile([NP, 3 * NI], f32)
            for kj in range(3):
                nc.sync.dma_start(out=cb[:, kj * NI:(kj + 1) * NI], in_=colbase[kj])

            # ---- per-tap index & weight fields ----
            wrap_k, w4_k = [], []
            for k in range(K):
                ki, kj = k // 3, k % 3
                dy = spool.tile([NP, NI], f32, tag="dy")
                dx = spool.tile([NP, NI], f32, tag="dx")
                nc.sync.dma_start(out=dy[:], in_=off[k, 0])
                nc.sync.dma_start(out=dx[:], in_=off[k, 1])

                py = spool.tile([NP, NI], f32, tag="py")
                px = spool.tile([NP, NI], f32, tag="px")
                # py = dy + (p - 1 + ki + PAD); px = dx + (i - 1 + kj + PAD)
                nc.vector.tensor_scalar(py[:], dy[:], rb[:, ki:ki + 1], None, add)
                nc.vector.tensor_tensor(px[:], dx[:], cb[:, kj * NI:(kj + 1) * NI], add)

                def floor_frac(src, tag):
                    ti = spool.tile([NP, NI], mybir.dt.int32, tag=f"ti{tag}")
                    tf = spool.tile([NP, NI], f32, tag=f"tf{tag}")
                    corr = spool.tile([NP, NI], f32, tag=f"co{tag}")
                    fl = spool.tile([NP, NI], f32, tag=f"fl{tag}")
                    fr = spool.tile([NP, NI], f32, tag=f"fr{tag}")
                    nc.vector.tensor_copy(ti[:], src[:])
                    nc.vector.tensor_copy(tf[:], ti[:])
                    nc.vector.tensor_tensor(corr[:], tf[:], src[:], is_gt)
                    nc.vector.tensor_tensor(fl[:], tf[:], corr[:], sub)
                    nc.vector.tensor_tensor(fr[:], src[:], fl[:], sub)
                    return fl, fr

                y0f, ly = floor_frac(py, "y")
                x0f, lx = floor_frac(px, "x")

                wy0 = spool.tile([NP, NI], f32, tag="wy0")
                wx0 = spool.tile([NP, NI], f32, tag="wx0")
                w00 = spool.tile([NP, NI], f32, tag="w00")
                w01 = spool.tile([NP, NI], f32, tag="w01")
                w10 = spool.tile([NP, NI], f32, tag="w10")
                w11 = spool.tile([NP, NI], f32, tag="w11")
                nc.vector.tensor_scalar(wy0[:], ly[:], 1.0, -1.0, sub, mult)
                nc.vector.tensor_scalar(wx0[:], lx[:], 1.0, -1.0, sub, mult)
                nc.vector.tensor_tensor(w00[:], wy0[:], wx0[:], mult)  # (xp0,yp0)
                nc.vector.tensor_tensor(w01[:], ly[:], wx0[:], mult)   # (xp0,yp1)
                nc.vector.tensor_tensor(w10[:], wy0[:], lx[:], mult)   # (xp1,yp0)
                nc.vector.tensor_tensor(w11[:], ly[:], lx[:], mult)    # (xp1,yp1)
                # packed per-tap corner weights W4[p, i, corner] (bf16)
                w4 = ppool.tile([NP, NI, 4], bf16, tag=f"w4_{k}")
                nc.vector.tensor_copy(w4[:, :, 0:1], w00[:, :, None])
                nc.vector.tensor_copy(w4[:, :, 1:2], w01[:, :, None])
                nc.vector.tensor_copy(w4[:, :, 2:3], w10[:, :, None])
                nc.vector.tensor_copy(w4[:, :, 3:4], w11[:, :, None])

                r1 = spool.tile([NP, NI], f32, tag="r1")
                r2 = spool.tile([NP, NI], f32, tag="r2")
                nc.vector.tensor_scalar(r1[:], y0f[:], float(TX), None, mult)
                nc.vector.tensor_tensor(r2[:], r1[:], x0f[:], add)
                nc.vector.tensor_scalar(r2[:], r2[:], float(TROWS - 2), 0.0, amin, amax)

                # ---- wrap r into dma_gather idx layout ----
                # wrapped[pp+16q, j*8+ph] = r[16ph+pp, j]
                rpsum = psp.tile([128, 128], f32, tag="rpsum", bufs=1)
                nc.tensor.transpose(out=rpsum[:], in_=r2[:], identity=identf[:])
                wrapP = psp.tile([128, 8 * 128], f32, tag="wrapP", bufs=1)
                for ph in range(8):
                    rT_ph = spool.tile([128, 128], f32, tag="rT", bufs=2,
                                       name=f"rT{ph}")
                    nc.vector.tensor_copy(
                        rT_ph[:].rearrange("p (q s) -> p q s", q=8),
                        rpsum[:, None, 16 * ph:16 * (ph + 1)]
                        .to_broadcast([128, 8, 16]))
                    nc.tensor.transpose(
                        out=wrapP[:, 128 * ph:128 * (ph + 1)],
                        in_=rT_ph[:],
                        identity=identf[:])
                wrapped = ppool.tile([128, 1024], i16, tag=f"wrap_{k}")
                nc.vector.tensor_copy(
                    wrapped[:].rearrange("p (j h) -> p j h", h=8),
                    wrapP[:].rearrange("p (h j) -> p j h", h=8))

                wrap_k.append(wrapped)
                w4_k.append(w4)

            # ---- main loop over i-chunks ----
            out_bf = ppool.tile([Cout, NP, NI], bf16, tag="outbf")  # [o, p, i]
            for cc in range(NCHUNK):
                i0 = cc * CHUNK
                vals = []
                for k in range(K):
                    G = gpool.tile([128, CHUNK * 256], bf16, tag="G")
                    nc.gpsimd.dma_gather(
                        out_ap=G[:].rearrange("p (j e) -> p j e", e=256),
                        in_ap=tbl_src,
                        idxs_ap=wrap_k[k][:, 128 * cc:128 * (cc + 1)],
                        num_idxs=CHUNK * 128,
                        num_idxs_reg=CHUNK * 128,
                        elem_size=256,
                        elem_step=128,
                        single_packet=False,
                        queue_num=(cc * K + k) % 4,
                    )
                    # weighted corners: one big mult, then 3 strided adds
                    # G view [p, i, corner(yp,xp), c]; W4 bcast over c
                    P = spool.tile([128, CHUNK, 4, Cout], bf16, tag="P", bufs=2)
                    nc.vector.tensor_tensor(
                        P[:],
                        G[:].rearrange("p (i k c) -> p i k c", k=4, c=Cout),
                        w4_k[k][:, i0:i0 + CHUNK, :, None]
                        .to_broadcast([128, CHUNK, 4, Cout]),
                        mult)
                    val = vpool.tile([128, CHUNK, Cout], bf16, tag=f"val{k}")
                    nc.vector.tensor_tensor(
                        val[:], P[:, :, 0, :], P[:, :, 1, :], add)
                    nc.vector.tensor_tensor(
                        val[:], val[:], P[:, :, 2, :], add)
                    nc.vector.tensor_tensor(
                        val[:], val[:], P[:, :, 3, :], add)
                    vals.append(val)

                # transpose + copy to valT
                valts = []
                for g in range(5):
                    vt = tpool.tile([128, CHUNK * 128], bf16, tag=f"vt{g}")
                    valts.append(vt)
                for i in range(CHUNK):
                    for g in range(5):
                        ka, kb = 2 * g, min(2 * g + 1, K - 1)
                        pt = psp.tile([128, 128], bf16, tag="pt", bufs=2)
                        nc.tensor.transpose(
                            out=pt[0:64, :],
                            in_=vals[ka][:, i, :],
                            identity=ident[:])
                        nc.tensor.transpose(
                            out=pt[64:128, :],
                            in_=vals[kb][:, i, :],
                            identity=ident[:])
                        nc.scalar.activation(
                            out=valts[g][:, i * 128:(i + 1) * 128],
                            in_=pt[:, :], func=ACopy)

                # conv matmuls + bias
                for isub in range(0, CHUNK, 2):
                    outps = [psp.tile([Cout, 128], f32, tag="op", bufs=3,
                                      name=f"op{t}") for t in range(2)]
                    for g in range(5):
                        for t in range(2):
                            i = isub + t
                            nc.tensor.matmul(
                                out=outps[t][:],
                                lhsT=w2sb[:, g * Cout:(g + 1) * Cout],
                                rhs=valts[g][:, i * 128:(i + 1) * 128],
                                start=(g == 0), stop=(g == 4))
                    for t in range(2):
                        ia = i0 + isub + t
                        nc.vector.tensor_scalar(
                            out_bf[:, :, ia], outps[t][:],
                            bias_sb[:, 0:1], None, add)

            # ---- cast + store ----
            for pc in range(8):
                cbuf = spool.tile([Cout, 16, NI], f32, tag="cast", bufs=2)
                nc.vector.tensor_copy(cbuf[:], out_bf[:, pc * 16:(pc + 1) * 16, :])
                nc.sync.dma_start(
                    out=outp[:, pc * 16 * NI:(pc + 1) * 16 * NI],
                    in_=cbuf[:])

    nc.compile()
    return nc


def _host_prep(x, offset, weight, bias):
    bf16 = ml_dtypes.bfloat16
    # per-core 4-corner tables (batch x W-half, with halo)
    tbls = []
    for core in range(8):
        b, w0 = core // 2, (core % 2) * WH
        T = np.zeros((TY, TX, 2, C), dtype=bf16)
        xlo = max(0, w0 - PAD)
        xhi = min(W, w0 + WH + PAD)
        # table x-col for global x: xt = x - w0 + PAD
        tlo, thi = xlo - w0 + PAD, xhi - w0 + PAD
        xt = np.ascontiguousarray(x[b].transpose(1, 2, 0))  # [H, W, C]
        T[PAD:PAD + H, tlo:thi, 0, :] = xt[:, xlo:xhi]
        T[PAD - 1:PAD - 1 + H, tlo:thi, 1, :] = xt[:, xlo:xhi]
        tbls.append(T.reshape(TROWS, 128))
    # conv weights: W2[g, ks*64+c, o] = weight[o, c, 2g+ks]
    wr = weight.reshape(Cout, C, K)
    w2 = np.zeros((5, 128, Cout), dtype=bf16)
    for g in range(5):
        w2[g, 0:64, :] = wr[:, :, 2 * g].T
        if 2 * g + 1 < K:
            w2[g, 64:128, :] = wr[:, :, 2 * g + 1].T
    biasc = np.ascontiguousarray(bias.reshape(Cout, 1).astype(np.float32))
    rowbase = np.zeros((NP, 3), np.float32)
    for ki in range(3):
        rowbase[:, ki] = np.arange(NP) - 1 + ki + PAD
    colbase = np.zeros((3, NP, NI), np.float32)
    for kj in range(3):
        colbase[kj, :, :] = (np.arange(NI) - 1 + kj + PAD)[None, :]
    return tbls, w2, biasc, rowbase, colbase


def kernel(x, offset, weight, bias):
    from concourse.bass_utils import run_bass_kernel_spmd

    assert float(np.abs(offset).max()) < PAD - 2.0, "offset outside supported band"

    if "nc" not in _CACHE:
        _CACHE["nc"] = _build_bass()
    nc = _CACHE["nc"]

    tbls, w2, biasc, rowbase, colbase = _host_prep(x, offset, weight, bias)

    in_maps = []
    for core in range(8):
        b, w0 = core // 2, (core % 2) * WH
        offs = np.ascontiguousarray(
            offset[b].reshape(K, 2, H, W)[:, :, :, w0:w0 + WH]).astype(np.float32)
        in_maps.append({
            "tbl": tbls[core],
            "off": offs,
            "w2": w2,
            "bias": biasc,
            "rowbase": rowbase,
            "colbase": colbase,
        })

    res = run_bass_kernel_spmd(nc, in_maps, list(range(8)))

    out = np.empty((B, Cout, H, W), np.float32)
    for core in range(8):
        b, w0 = core // 2, (core % 2) * WH
        out[b, :, :, w0:w0 + WH] = res.results[core]["out"].reshape(Cout, NP, NI)
    return out

